# revision 12
# baseline (speedup 1.0000x reference)
"""Llama decode attention (B=16, S=1, DIM=4096, NH=32, NKV=8, HD=128,
kv_len=4097) on 8 trn2 NeuronCores, tensor-parallel over kv-heads.

Per core c: kv head c, q heads 4c..4c+3.
Host folds RoPE + 1/sqrt(HD) into wq (RoPE into wk), pre-transposes all
weights, and shards the KV cache by head into contiguous slices.
Device computes scoresT = K^T-blocks^T @ qT directly in the [kv, h]
orientation, exp (unnormalized), PV accumulation, softmax denominators
via a ones-matmul over partitions, normalization as a single diag-matmul,
then the row-parallel wo matmul. Host sums the 8 partial outputs.
"""

import numpy as np
from contextlib import ExitStack

from concourse import bass, bacc, tile, mybir, masks
from concourse.bass_utils import run_bass_kernel_spmd

F32 = mybir.dt.float32

B = 16
DIM = 4096
NH = 32
NKV = 8
HD = 128
NREP = NH // NKV          # 4 q heads per kv head (per core)
START = 4096              # static start_pos
L = START                 # cached positions
NB = L // 128             # 32 kv blocks of 128
NCORES = 8
DQ = NREP * HD            # 512 local q dim
KB = 34                   # probsT col-blocks: 32 cached + 1 new + 1 pad(unused)

LAST_EXEC_NS = None
LAST_RESULTS = None

_NC_CACHE = {}


def _build_kernel(nc):
    # ---- DRAM I/O (per-core shard layouts, prepared on host) ----
    xt_d = nc.dram_tensor("xt", [128, 32 * 16], F32, kind="ExternalInput")
    wqt_d = nc.dram_tensor("wqt", [128, 32 * 512], F32, kind="ExternalInput")
    wkt_d = nc.dram_tensor("wkt", [128, 32 * 128], F32, kind="ExternalInput")
    wvt_d = nc.dram_tensor("wvt", [128, 32 * 128], F32, kind="ExternalInput")
    wot_d = nc.dram_tensor("wot", [128, 4 * 4096], F32, kind="ExternalInput")
    kc_d = nc.dram_tensor("kc", [B, L, HD], F32, kind="ExternalInput")
    vc_d = nc.dram_tensor("vc", [B, L, HD], F32, kind="ExternalInput")
    y_d = nc.dram_tensor("y", [B, DIM], F32, kind="ExternalOutput")

    with tile.TileContext(nc) as tc, ExitStack() as ctx:
        const_p = ctx.enter_context(tc.tile_pool(name="const", bufs=1))
        small_p = ctx.enter_context(tc.tile_pool(name="small", bufs=1))
        big_p = ctx.enter_context(tc.tile_pool(name="big", bufs=4))
        keyst_p = ctx.enter_context(tc.tile_pool(name="keyst", bufs=2))
        wo_p = ctx.enter_context(tc.tile_pool(name="wo", bufs=6))
        PS = bass.MemorySpace.PSUM
        mm_ps = ctx.enter_context(tc.tile_pool(name="mm_ps", bufs=2, space=PS))
        sc_ps = ctx.enter_context(tc.tile_pool(name="sc_ps", bufs=4, space=PS))
        at_ps = ctx.enter_context(tc.tile_pool(name="at_ps", bufs=1, space=PS))

        ident = const_p.tile([128, 128], F32)
        masks.make_identity(nc, ident[:])
        ones = const_p.tile([128, 1], F32)
        nc.gpsimd.memset(ones[:], 1.0)
        zb = const_p.tile([128, 1], F32)
        nc.gpsimd.memset(zb[:], 0.0)

        # persistent sbuf tensors
        xt_sb = small_p.tile([128, 512], F32)
        qT = small_p.tile([128, 64], F32)       # col = 4*b + h
        kTnew = small_p.tile([128, 16], F32)    # col = b
        xq_sb = small_p.tile([16, 512], F32)
        xk_sb = small_p.tile([16, 128], F32)
        xv_sb = small_p.tile([16, 128], F32)
        vrow = small_p.tile([1, B * HD], F32)   # new v, row layout
        probsT = small_p.tile([128, KB * 64], F32)  # unnormalized exp(scores)T
        recip = small_p.tile([64, 1], F32)
        diag = small_p.tile([64, 64], F32)
        aun_sb = small_p.tile([128, 64], F32)
        an_sb = small_p.tile([64, 128], F32)
        as_sb = small_p.tile([64, 128], F32)
        attn_sb = small_p.tile([128, 64], F32)
        y_sb = small_p.tile([16, 4096], F32)

        nc.scalar.dma_start(out=xt_sb[:], in_=xt_d[:, :])

        # ---- projections: xq = x @ wq^T etc. (weights stream as moving rhs)
        ps_xq = mm_ps.tile([16, 512], F32, tag="mm")
        for t in range(4):
            wt = big_p.tile([128, 4096], F32, tag="big")
            nc.scalar.dma_start(out=wt[:], in_=wqt_d[:, 4096 * t:4096 * (t + 1)])
            for kk in range(8):
                k = 8 * t + kk
                nc.tensor.matmul(
                    ps_xq[:, :],
                    xt_sb[:, 16 * k:16 * (k + 1)],
                    wt[:, 512 * kk:512 * (kk + 1)],
                    start=(k == 0), stop=(k == 31),
                )
        nc.scalar.copy(xq_sb[:], ps_xq[:])

        ps_xk = mm_ps.tile([16, 128], F32, tag="mm")
        wt = big_p.tile([128, 4096], F32, tag="big")
        nc.scalar.dma_start(out=wt[:], in_=wkt_d[:, :])
        for k in range(32):
            nc.tensor.matmul(
                ps_xk[:, :],
                xt_sb[:, 16 * k:16 * (k + 1)],
                wt[:, 128 * k:128 * (k + 1)],
                start=(k == 0), stop=(k == 31),
            )
        nc.scalar.copy(xk_sb[:], ps_xk[:])

        ps_xv = mm_ps.tile([16, 128], F32, tag="mm")
        wt = big_p.tile([128, 4096], F32, tag="big")
        nc.scalar.dma_start(out=wt[:], in_=wvt_d[:, :])
        for k in range(32):
            nc.tensor.matmul(
                ps_xv[:, :],
                xt_sb[:, 16 * k:16 * (k + 1)],
                wt[:, 128 * k:128 * (k + 1)],
                start=(k == 0), stop=(k == 31),
            )
        nc.scalar.copy(xv_sb[:], ps_xv[:])

        # ---- transposes of the small projections
        qT_v = qT[:].rearrange("p (b h) -> p h b", h=4)
        for h in range(4):
            ps_t = mm_ps.tile([128, 16], F32, tag="mm")
            nc.tensor.transpose(
                ps_t[:, :], xq_sb[:, 128 * h:128 * (h + 1)], ident[0:16, 0:16]
            )
            nc.vector.tensor_copy(qT_v[:, h, :], ps_t[:, :])
        ps_t = mm_ps.tile([128, 16], F32, tag="mm")
        nc.tensor.transpose(ps_t[:, :], xk_sb[:, :], ident[0:16, 0:16])
        nc.vector.tensor_copy(kTnew[:], ps_t[:, :])

        # new v into single-partition row layout (DMA can cross partitions)
        nc.scalar.dma_start(out=vrow[0:1, :], in_=xv_sb[:])

        # ---- new-token scores (kv position 4096), block KB-2 of probsT
        ps_sn = sc_ps.tile([128, 64], F32, tag="sc")
        for b in range(B):
            nc.tensor.matmul(
                ps_sn[0:1, 4 * b:4 * b + 4],
                kTnew[:, b:b + 1],
                qT[:, 4 * b:4 * b + 4],
                start=True, stop=True,
            )
        nc.scalar.activation(
            probsT[0:1, 64 * 32:64 * 32 + 64],
            ps_sn[0:1, 0:64],
            mybir.ActivationFunctionType.Exp,
            bias=zb[0:1, :],
        )

        # ---- wo weight loads for half 0, early, on the gpsimd (SWDGE) ring
        def load_wo_half(half):
            tiles = []
            for h in range(4):
                wot = wo_p.tile([128, 2048], F32, tag="wo")
                nc.gpsimd.dma_start(
                    out=wot[:],
                    in_=wot_d[:, 4096 * h + 2048 * half:
                              4096 * h + 2048 * (half + 1)],
                )
                tiles.append(wot)
            return tiles

        wo_half0 = load_wo_half(0)

        # ---- main streaming loop over batches
        attn_ps = at_ps.tile([128, 64], F32)
        probsT_v = probsT[:].rearrange("p (j c) -> p j c", c=64)
        for b in range(B):
            knat = big_p.tile([128, L], F32, tag="big")
            nc.sync.dma_start(
                out=knat[:].rearrange("p (j d) -> p j d", j=NB),
                in_=kc_d[b].rearrange("(j p) d -> p j d", p=128),
            )
            knat_v = knat[:].rearrange("p (j d) -> p j d", j=NB)
            keysT = keyst_p.tile([128, L], F32, tag="kT")
            for g in range(8):
                kt_ps = sc_ps.tile([128, 512], F32, tag="sc")
                for jj in range(4):
                    j = 4 * g + jj
                    nc.tensor.transpose(
                        kt_ps[:, 128 * jj:128 * (jj + 1)], knat_v[:, j, :],
                        ident[:, :],
                    )
                eng = nc.vector if g % 3 != 2 else nc.scalar
                if eng is nc.vector:
                    nc.vector.tensor_copy(
                        keysT[:, 512 * g:512 * (g + 1)], kt_ps[:, :])
                else:
                    nc.scalar.copy(keysT[:, 512 * g:512 * (g + 1)], kt_ps[:, :])

            vnat = big_p.tile([128, L], F32, tag="big")
            nc.scalar.dma_start(
                out=vnat[:].rearrange("p (j d) -> p j d", j=NB),
                in_=vc_d[b].rearrange("(j p) d -> p j d", p=128),
            )

            for g in range(4):
                ps_s = sc_ps.tile([128, 32], F32, tag="sc")
                for jj in range(8):
                    j = 8 * g + jj
                    nc.tensor.matmul(
                        ps_s[:, 4 * jj:4 * (jj + 1)],
                        keysT[:, 128 * j:128 * (j + 1)],
                        qT[:, 4 * b:4 * b + 4],
                        start=True, stop=True,
                    )
                nc.scalar.activation(
                    probsT_v[:, 8 * g:8 * (g + 1), 4 * b:4 * b + 4],
                    ps_s[:].rearrange("p (j c) -> p j c", c=4),
                    mybir.ActivationFunctionType.Exp,
                    bias=zb[:, :],
                )

            vnat_v = vnat[:].rearrange("p (j d) -> p j d", j=NB)
            for j in range(NB):
                nc.tensor.matmul(
                    attn_ps[:, 4 * b:4 * b + 4],
                    vnat_v[:, j, :],
                    probsT[:, 64 * j + 4 * b:64 * j + 4 * b + 4],
                    start=(j == 0), stop=False,
                )
            nc.tensor.matmul(
                attn_ps[:, 4 * b:4 * b + 4],
                vrow[0:1, HD * b:HD * (b + 1)],
                probsT[0:1, 64 * 32 + 4 * b:64 * 32 + 4 * b + 4],
                start=False, stop=True,
            )

        # ---- softmax denominators: ones-matmul over kv partitions
        ps_sum = mm_ps.tile([64, 1], F32, tag="mm")
        for j in range(NB):
            nc.tensor.matmul(
                ps_sum[:, :],
                probsT[:, 64 * j:64 * (j + 1)],
                ones[:, :],
                start=(j == 0), stop=False,
            )
        nc.tensor.matmul(
            ps_sum[:, :],
            probsT[0:1, 64 * 32:64 * 32 + 64],
            ones[0:1, :],
            start=False, stop=True,
        )
        nc.vector.reciprocal(recip[:], ps_sum[:, :])
        nc.vector.tensor_scalar_mul(diag[:], ident[0:64, 0:64], recip[:])

        # ---- normalize: attn = (attnT_un)^T scaled per (b,h), back to T
        nc.vector.tensor_copy(aun_sb[:], attn_ps[:, :])
        ps_an = mm_ps.tile([64, 128], F32, tag="mm")
        nc.tensor.transpose(ps_an[:, :], aun_sb[:], ident[:, :])
        nc.vector.tensor_copy(an_sb[:], ps_an[:, :])
        ps_as = mm_ps.tile([64, 128], F32, tag="mm")
        nc.tensor.matmul(ps_as[:, :], diag[:], an_sb[:], start=True, stop=True)
        nc.vector.tensor_copy(as_sb[:], ps_as[:, :])
        ps_at = mm_ps.tile([128, 64], F32, tag="mm")
        nc.tensor.transpose(ps_at[:, :], as_sb[:], ident[0:64, 0:64])
        nc.vector.tensor_copy(attn_sb[:], ps_at[:, :])

        # ---- y = attn @ wo_c^T   (contraction over local 512 dim)
        attn_v = attn_sb[:].rearrange("p (b h) -> p h b", h=4)
        for half in range(2):
            wo_tiles = wo_half0 if half == 0 else load_wo_half(1)
            for n in range(4):
                ps_y = mm_ps.tile([16, 512], F32, tag="mm")
                for h in range(4):
                    nc.tensor.matmul(
                        ps_y[:, :],
                        attn_v[:, h, :],
                        wo_tiles[h][:, 512 * n:512 * (n + 1)],
                        start=(h == 0), stop=(h == 3),
                    )
                col = 2048 * half + 512 * n
                nc.vector.tensor_copy(y_sb[:, col:col + 512], ps_y[:, :])
        nc.scalar.dma_start(out=y_d[:, :], in_=y_sb[:])

    nc.compile()
    return nc


def _get_nc():
    if "nc" not in _NC_CACHE:
        nc = bacc.Bacc("TRN2", target_bir_lowering=False, debug=False)
        _NC_CACHE["nc"] = _build_kernel(nc)
    return _NC_CACHE["nc"]


def _prep_inputs(x, freqs_cos, freqs_sin, cache_k, cache_v, wq, wk, wv, wo):
    """Host-side sharding + layout prep. Returns per-core in_maps."""
    x2 = np.ascontiguousarray(np.asarray(x, np.float32).reshape(B, DIM))
    cos = np.asarray(freqs_cos, np.float32).reshape(HD // 2)
    sin = np.asarray(freqs_sin, np.float32).reshape(HD // 2)
    wq = np.asarray(wq, np.float32)
    wk = np.asarray(wk, np.float32)
    wv = np.asarray(wv, np.float32)
    wo = np.asarray(wo, np.float32)
    ck = np.asarray(cache_k, np.float32)
    cv = np.asarray(cache_v, np.float32)

    def rope_fold(w, nheads):
        w4 = w.reshape(nheads, HD // 2, 2, DIM)
        out = np.empty_like(w4)
        c = cos[None, :, None]
        s = sin[None, :, None]
        out[:, :, 0, :] = c * w4[:, :, 0, :] - s * w4[:, :, 1, :]
        out[:, :, 1, :] = s * w4[:, :, 0, :] + c * w4[:, :, 1, :]
        return out.reshape(nheads * HD, DIM)

    scale = np.float32(1.0 / np.sqrt(HD).astype(np.float32))
    wq_f = rope_fold(wq, NH) * scale
    wk_f = rope_fold(wk, NKV)

    xt = np.ascontiguousarray(
        x2.T.reshape(32, 128, 16).transpose(1, 0, 2).reshape(128, 512))

    in_maps = []
    for c in range(NCORES):
        wq_c = wq_f[DQ * c:DQ * (c + 1)]                       # [512, 4096]
        wqt = wq_c.T.reshape(32, 128, 512).transpose(1, 0, 2).reshape(128, 32 * 512)
        wk_c = wk_f[HD * c:HD * (c + 1)]                       # [128, 4096]
        wkt = wk_c.T.reshape(32, 128, 128).transpose(1, 0, 2).reshape(128, 32 * 128)
        wv_c = wv[HD * c:HD * (c + 1)]
        wvt = wv_c.T.reshape(32, 128, 128).transpose(1, 0, 2).reshape(128, 32 * 128)
        wo_c = wo[:, DQ * c:DQ * (c + 1)]                      # [4096, 512]
        wot = wo_c.T.reshape(4, 128, 4096).transpose(1, 0, 2).reshape(128, 4 * 4096)
        kc = ck[:, :L, c, :]                                    # [B,L,128]
        vc = cv[:, :L, c, :]                                    # [B,L,128]
        in_maps.append({
            "xt": xt,
            "wqt": np.ascontiguousarray(wqt),
            "wkt": np.ascontiguousarray(wkt),
            "wvt": np.ascontiguousarray(wvt),
            "wot": np.ascontiguousarray(wot),
            "kc": np.ascontiguousarray(kc),
            "vc": np.ascontiguousarray(vc),
        })
    return in_maps


def kernel(x, start_pos, freqs_cos, freqs_sin, cache_k, cache_v, wq, wk, wv, wo):
    global LAST_EXEC_NS, LAST_RESULTS
    assert int(start_pos) == START, f"kernel hardcodes start_pos={START}"
    nc = _get_nc()
    in_maps = _prep_inputs(x, freqs_cos, freqs_sin, cache_k, cache_v,
                           wq, wk, wv, wo)
    res = run_bass_kernel_spmd(nc, in_maps, core_ids=list(range(NCORES)))
    LAST_EXEC_NS = res.exec_time_ns
    LAST_RESULTS = res
    y = np.zeros((B, DIM), np.float32)
    for c in range(NCORES):
        y += res.results[c]["y"]
    return y.reshape(B, 1, DIM)


# revision 16
# speedup vs baseline: 1.0257x; 1.0257x over previous
"""Llama decode attention (B=16, S=1, DIM=4096, NH=32, NKV=8, HD=128,
kv_len=4097) on 8 trn2 NeuronCores, tensor-parallel over kv-heads.

Per core c: kv head c, q heads 4c..4c+3.
Host folds RoPE + 1/sqrt(HD) into wq (RoPE into wk), pre-transposes all
weights, and shards the KV cache by head into contiguous slices.
Device computes scoresT = K^T-blocks^T @ qT directly in the [kv, h]
orientation, exp (unnormalized), PV accumulation, softmax denominators
via a ones-matmul over partitions, normalization as a single diag-matmul,
then the row-parallel wo matmul. Host sums the 8 partial outputs.
"""

import numpy as np
from contextlib import ExitStack

from concourse import bass, bacc, tile, mybir, masks
from concourse.bass_utils import run_bass_kernel_spmd

F32 = mybir.dt.float32

B = 16
DIM = 4096
NH = 32
NKV = 8
HD = 128
NREP = NH // NKV          # 4 q heads per kv head (per core)
START = 4096              # static start_pos
L = START                 # cached positions
NB = L // 128             # 32 kv blocks of 128
NCORES = 8
DQ = NREP * HD            # 512 local q dim
KB = 34                   # probsT col-blocks: 32 cached + 1 new + 1 pad(unused)

LAST_EXEC_NS = None
LAST_RESULTS = None

_NC_CACHE = {}


def _build_kernel(nc):
    # ---- DRAM I/O (per-core shard layouts, prepared on host) ----
    xt_d = nc.dram_tensor("xt", [128, 32 * 16], F32, kind="ExternalInput")
    wqt_d = nc.dram_tensor("wqt", [128, 32 * 512], F32, kind="ExternalInput")
    wkt_d = nc.dram_tensor("wkt", [128, 32 * 128], F32, kind="ExternalInput")
    wvt_d = nc.dram_tensor("wvt", [128, 32 * 128], F32, kind="ExternalInput")
    wot_d = nc.dram_tensor("wot", [128, 4 * 4096], F32, kind="ExternalInput")
    kc_d = nc.dram_tensor("kc", [B, L, HD], F32, kind="ExternalInput")
    vc_d = nc.dram_tensor("vc", [B, L, HD], F32, kind="ExternalInput")
    y_d = nc.dram_tensor("y", [B, DIM], F32, kind="ExternalOutput")

    with tile.TileContext(nc) as tc, ExitStack() as ctx:
        const_p = ctx.enter_context(tc.tile_pool(name="const", bufs=1))
        small_p = ctx.enter_context(tc.tile_pool(name="small", bufs=1))
        big_p = ctx.enter_context(tc.tile_pool(name="big", bufs=5))
        keyst_p = ctx.enter_context(tc.tile_pool(name="keyst", bufs=2))
        wo_p = ctx.enter_context(tc.tile_pool(name="wo", bufs=6))
        PS = bass.MemorySpace.PSUM
        mm_ps = ctx.enter_context(tc.tile_pool(name="mm_ps", bufs=2, space=PS))
        sc_ps = ctx.enter_context(tc.tile_pool(name="sc_ps", bufs=4, space=PS))
        at_ps = ctx.enter_context(tc.tile_pool(name="at_ps", bufs=1, space=PS))

        ident = const_p.tile([128, 128], F32)
        masks.make_identity(nc, ident[:])
        ones = const_p.tile([128, 1], F32)
        nc.gpsimd.memset(ones[:], 1.0)
        zb = const_p.tile([128, 1], F32)
        nc.gpsimd.memset(zb[:], 0.0)

        # persistent sbuf tensors
        xt_sb = small_p.tile([128, 512], F32)
        qT = small_p.tile([128, 64], F32)       # col = 4*b + h
        kTnew = small_p.tile([128, 16], F32)    # col = b
        xq_sb = small_p.tile([16, 512], F32)
        xk_sb = small_p.tile([16, 128], F32)
        xv_sb = small_p.tile([16, 128], F32)
        vrow = small_p.tile([1, B * HD], F32)   # new v, row layout
        probsT = small_p.tile([128, KB * 64], F32)  # unnormalized exp(scores)T
        recip = small_p.tile([64, 1], F32)
        diag = small_p.tile([64, 64], F32)
        aun_sb = small_p.tile([128, 64], F32)
        an_sb = small_p.tile([64, 128], F32)
        as_sb = small_p.tile([64, 128], F32)
        attn_sb = small_p.tile([128, 64], F32)
        y_sb = small_p.tile([16, 4096], F32)

        nc.scalar.dma_start(out=xt_sb[:], in_=xt_d[:, :])

        # ---- projections: xq = x @ wq^T etc. (weights stream as moving rhs)
        ps_xq = mm_ps.tile([16, 512], F32, tag="mm")
        for t in range(4):
            wt = big_p.tile([128, 4096], F32, tag="big")
            nc.gpsimd.dma_start(out=wt[:], in_=wqt_d[:, 4096 * t:4096 * (t + 1)])
            for kk in range(8):
                k = 8 * t + kk
                nc.tensor.matmul(
                    ps_xq[:, :],
                    xt_sb[:, 16 * k:16 * (k + 1)],
                    wt[:, 512 * kk:512 * (kk + 1)],
                    start=(k == 0), stop=(k == 31),
                )
        nc.scalar.copy(xq_sb[:], ps_xq[:])

        ps_xk = mm_ps.tile([16, 128], F32, tag="mm")
        wt = big_p.tile([128, 4096], F32, tag="big")
        nc.gpsimd.dma_start(out=wt[:], in_=wkt_d[:, :])
        for k in range(32):
            nc.tensor.matmul(
                ps_xk[:, :],
                xt_sb[:, 16 * k:16 * (k + 1)],
                wt[:, 128 * k:128 * (k + 1)],
                start=(k == 0), stop=(k == 31),
            )
        nc.scalar.copy(xk_sb[:], ps_xk[:])

        ps_xv = mm_ps.tile([16, 128], F32, tag="mm")
        wt = big_p.tile([128, 4096], F32, tag="big")
        nc.gpsimd.dma_start(out=wt[:], in_=wvt_d[:, :])
        for k in range(32):
            nc.tensor.matmul(
                ps_xv[:, :],
                xt_sb[:, 16 * k:16 * (k + 1)],
                wt[:, 128 * k:128 * (k + 1)],
                start=(k == 0), stop=(k == 31),
            )
        nc.scalar.copy(xv_sb[:], ps_xv[:])

        # ---- transposes of the small projections
        qT_v = qT[:].rearrange("p (b h) -> p h b", h=4)
        for h in range(4):
            ps_t = mm_ps.tile([128, 16], F32, tag="mm")
            nc.tensor.transpose(
                ps_t[:, :], xq_sb[:, 128 * h:128 * (h + 1)], ident[0:16, 0:16]
            )
            nc.vector.tensor_copy(qT_v[:, h, :], ps_t[:, :])
        ps_t = mm_ps.tile([128, 16], F32, tag="mm")
        nc.tensor.transpose(ps_t[:, :], xk_sb[:, :], ident[0:16, 0:16])
        nc.vector.tensor_copy(kTnew[:], ps_t[:, :])

        # new v into single-partition row layout (DMA can cross partitions)
        nc.scalar.dma_start(out=vrow[0:1, :], in_=xv_sb[:])

        # ---- new-token scores (kv position 4096), block KB-2 of probsT
        ps_sn = sc_ps.tile([128, 64], F32, tag="sc")
        for b in range(B):
            nc.tensor.matmul(
                ps_sn[0:1, 4 * b:4 * b + 4],
                kTnew[:, b:b + 1],
                qT[:, 4 * b:4 * b + 4],
                start=True, stop=True,
            )
        nc.scalar.activation(
            probsT[0:1, 64 * 32:64 * 32 + 64],
            ps_sn[0:1, 0:64],
            mybir.ActivationFunctionType.Exp,
            bias=zb[0:1, :],
        )

        # ---- wo weight loads for half 0, early, on the gpsimd (SWDGE) ring
        def load_wo_half(half):
            tiles = []
            for h in range(4):
                wot = wo_p.tile([128, 2048], F32, tag="wo")
                nc.gpsimd.dma_start(
                    out=wot[:],
                    in_=wot_d[:, 4096 * h + 2048 * half:
                              4096 * h + 2048 * (half + 1)],
                )
                tiles.append(wot)
            return tiles

        wo_half0 = load_wo_half(0)

        # ---- main streaming loop over batches
        attn_ps = at_ps.tile([128, 64], F32)
        probsT_v = probsT[:].rearrange("p (j c) -> p j c", c=64)
        for b in range(B):
            knat = big_p.tile([128, L], F32, tag="big")
            nc.sync.dma_start(
                out=knat[:].rearrange("p (j d) -> p j d", j=NB),
                in_=kc_d[b].rearrange("(j p) d -> p j d", p=128),
            )
            knat_v = knat[:].rearrange("p (j d) -> p j d", j=NB)
            keysT = keyst_p.tile([128, L], F32, tag="kT")
            for g in range(8):
                kt_ps = sc_ps.tile([128, 512], F32, tag="sc")
                for jj in range(4):
                    j = 4 * g + jj
                    nc.tensor.transpose(
                        kt_ps[:, 128 * jj:128 * (jj + 1)], knat_v[:, j, :],
                        ident[:, :],
                    )
                eng = nc.vector if g % 3 != 2 else nc.scalar
                if eng is nc.vector:
                    nc.vector.tensor_copy(
                        keysT[:, 512 * g:512 * (g + 1)], kt_ps[:, :])
                else:
                    nc.scalar.copy(keysT[:, 512 * g:512 * (g + 1)], kt_ps[:, :])

            vnat = big_p.tile([128, L], F32, tag="big")
            nc.scalar.dma_start(
                out=vnat[:].rearrange("p (j d) -> p j d", j=NB),
                in_=vc_d[b].rearrange("(j p) d -> p j d", p=128),
            )

            for g in range(4):
                ps_s = sc_ps.tile([128, 32], F32, tag="sc")
                for jj in range(8):
                    j = 8 * g + jj
                    nc.tensor.matmul(
                        ps_s[:, 4 * jj:4 * (jj + 1)],
                        keysT[:, 128 * j:128 * (j + 1)],
                        qT[:, 4 * b:4 * b + 4],
                        start=True, stop=True,
                    )
                nc.scalar.activation(
                    probsT_v[:, 8 * g:8 * (g + 1), 4 * b:4 * b + 4],
                    ps_s[:].rearrange("p (j c) -> p j c", c=4),
                    mybir.ActivationFunctionType.Exp,
                    bias=zb[:, :],
                )

            vnat_v = vnat[:].rearrange("p (j d) -> p j d", j=NB)
            for j in range(NB):
                nc.tensor.matmul(
                    attn_ps[:, 4 * b:4 * b + 4],
                    vnat_v[:, j, :],
                    probsT[:, 64 * j + 4 * b:64 * j + 4 * b + 4],
                    start=(j == 0), stop=False,
                )
            nc.tensor.matmul(
                attn_ps[:, 4 * b:4 * b + 4],
                vrow[0:1, HD * b:HD * (b + 1)],
                probsT[0:1, 64 * 32 + 4 * b:64 * 32 + 4 * b + 4],
                start=False, stop=True,
            )

        # ---- softmax denominators: ones-matmul over kv partitions
        ps_sum = mm_ps.tile([64, 1], F32, tag="mm")
        for j in range(NB):
            nc.tensor.matmul(
                ps_sum[:, :],
                probsT[:, 64 * j:64 * (j + 1)],
                ones[:, :],
                start=(j == 0), stop=False,
            )
        nc.tensor.matmul(
            ps_sum[:, :],
            probsT[0:1, 64 * 32:64 * 32 + 64],
            ones[0:1, :],
            start=False, stop=True,
        )
        nc.vector.reciprocal(recip[:], ps_sum[:, :])
        nc.vector.tensor_scalar_mul(diag[:], ident[0:64, 0:64], recip[:])

        # ---- normalize: attn = (attnT_un)^T scaled per (b,h), back to T
        nc.vector.tensor_copy(aun_sb[:], attn_ps[:, :])
        ps_an = mm_ps.tile([64, 128], F32, tag="mm")
        nc.tensor.transpose(ps_an[:, :], aun_sb[:], ident[:, :])
        nc.vector.tensor_copy(an_sb[:], ps_an[:, :])
        ps_as = mm_ps.tile([64, 128], F32, tag="mm")
        nc.tensor.matmul(ps_as[:, :], diag[:], an_sb[:], start=True, stop=True)
        nc.vector.tensor_copy(as_sb[:], ps_as[:, :])
        ps_at = mm_ps.tile([128, 64], F32, tag="mm")
        nc.tensor.transpose(ps_at[:, :], as_sb[:], ident[0:64, 0:64])
        nc.vector.tensor_copy(attn_sb[:], ps_at[:, :])

        # ---- y = attn @ wo_c^T   (contraction over local 512 dim)
        attn_v = attn_sb[:].rearrange("p (b h) -> p h b", h=4)
        for half in range(2):
            wo_tiles = wo_half0 if half == 0 else load_wo_half(1)
            for n in range(4):
                ps_y = mm_ps.tile([16, 512], F32, tag="mm")
                for h in range(4):
                    nc.tensor.matmul(
                        ps_y[:, :],
                        attn_v[:, h, :],
                        wo_tiles[h][:, 512 * n:512 * (n + 1)],
                        start=(h == 0), stop=(h == 3),
                    )
                col = 2048 * half + 512 * n
                nc.vector.tensor_copy(y_sb[:, col:col + 512], ps_y[:, :])
        nc.scalar.dma_start(out=y_d[:, :], in_=y_sb[:])

    nc.compile()
    return nc


def _get_nc():
    if "nc" not in _NC_CACHE:
        nc = bacc.Bacc("TRN2", target_bir_lowering=False, debug=False)
        _NC_CACHE["nc"] = _build_kernel(nc)
    return _NC_CACHE["nc"]


def _prep_inputs(x, freqs_cos, freqs_sin, cache_k, cache_v, wq, wk, wv, wo):
    """Host-side sharding + layout prep. Returns per-core in_maps."""
    x2 = np.ascontiguousarray(np.asarray(x, np.float32).reshape(B, DIM))
    cos = np.asarray(freqs_cos, np.float32).reshape(HD // 2)
    sin = np.asarray(freqs_sin, np.float32).reshape(HD // 2)
    wq = np.asarray(wq, np.float32)
    wk = np.asarray(wk, np.float32)
    wv = np.asarray(wv, np.float32)
    wo = np.asarray(wo, np.float32)
    ck = np.asarray(cache_k, np.float32)
    cv = np.asarray(cache_v, np.float32)

    def rope_fold(w, nheads):
        w4 = w.reshape(nheads, HD // 2, 2, DIM)
        out = np.empty_like(w4)
        c = cos[None, :, None]
        s = sin[None, :, None]
        out[:, :, 0, :] = c * w4[:, :, 0, :] - s * w4[:, :, 1, :]
        out[:, :, 1, :] = s * w4[:, :, 0, :] + c * w4[:, :, 1, :]
        return out.reshape(nheads * HD, DIM)

    scale = np.float32(1.0 / np.sqrt(HD).astype(np.float32))
    wq_f = rope_fold(wq, NH) * scale
    wk_f = rope_fold(wk, NKV)

    xt = np.ascontiguousarray(
        x2.T.reshape(32, 128, 16).transpose(1, 0, 2).reshape(128, 512))

    in_maps = []
    for c in range(NCORES):
        wq_c = wq_f[DQ * c:DQ * (c + 1)]                       # [512, 4096]
        wqt = wq_c.T.reshape(32, 128, 512).transpose(1, 0, 2).reshape(128, 32 * 512)
        wk_c = wk_f[HD * c:HD * (c + 1)]                       # [128, 4096]
        wkt = wk_c.T.reshape(32, 128, 128).transpose(1, 0, 2).reshape(128, 32 * 128)
        wv_c = wv[HD * c:HD * (c + 1)]
        wvt = wv_c.T.reshape(32, 128, 128).transpose(1, 0, 2).reshape(128, 32 * 128)
        wo_c = wo[:, DQ * c:DQ * (c + 1)]                      # [4096, 512]
        wot = wo_c.T.reshape(4, 128, 4096).transpose(1, 0, 2).reshape(128, 4 * 4096)
        kc = ck[:, :L, c, :]                                    # [B,L,128]
        vc = cv[:, :L, c, :]                                    # [B,L,128]
        in_maps.append({
            "xt": xt,
            "wqt": np.ascontiguousarray(wqt),
            "wkt": np.ascontiguousarray(wkt),
            "wvt": np.ascontiguousarray(wvt),
            "wot": np.ascontiguousarray(wot),
            "kc": np.ascontiguousarray(kc),
            "vc": np.ascontiguousarray(vc),
        })
    return in_maps


def kernel(x, start_pos, freqs_cos, freqs_sin, cache_k, cache_v, wq, wk, wv, wo):
    global LAST_EXEC_NS, LAST_RESULTS
    assert int(start_pos) == START, f"kernel hardcodes start_pos={START}"
    nc = _get_nc()
    in_maps = _prep_inputs(x, freqs_cos, freqs_sin, cache_k, cache_v,
                           wq, wk, wv, wo)
    res = run_bass_kernel_spmd(nc, in_maps, core_ids=list(range(NCORES)))
    LAST_EXEC_NS = res.exec_time_ns
    LAST_RESULTS = res
    y = np.zeros((B, DIM), np.float32)
    for c in range(NCORES):
        y += res.results[c]["y"]
    return y.reshape(B, 1, DIM)
